# revision 21
# baseline (speedup 1.0000x reference)
"""BalancedMoE (B=8192, D=2048, E=8, top-2) on 8 Trainium2 NeuronCores.

Strategy: expert-parallel with host-side sparse dispatch.
  - Host computes gate logits / top-2 routing / softmax gates, gathers each
    expert's tokens into a k-tiled [128, KT, C] layout (contiguous 16KB
    per-partition DMA runs), in bf16.
  - Core e runs a dense [C, D] x [D, D] matmul for expert e only
    (top-2 of 8 experts => 4x less FLOPs than the dense reference),
    with the expert weight matrix stationary in SBUF.
  - Host scatters the per-expert outputs back and combines with the
    gate weights in fp32.

Per-core Bass kernel: outT[o, t] = sum_d W_e[o, d] * toks[t, d] + b_e[o]
  lhsT = W_e^T tiles (stationary), rhs = token tiles (moving).

bf16 inputs halve HBM traffic vs fp32 (PE rate is identical: 1 cycle/row
for both bf16 and float32r at >=256 columns); the DMA-bound startup and
weight-stream phases shrink accordingly. Accumulation stays fp32 in PSUM.
"""

import os

import numpy as np

P = 128
B = 8192
D_LAT = 1024
D_EMB = 1024
D = D_LAT + D_EMB  # 2048
E = 8
TOPK = 2
N_CORES = 8


# ----------------------------------------------------------------- device ---

_cache = {}


def _ntff_shim():
    """Register the axon NTFF profile hook that the boot skips when
    antenv.axon_hooks is missing (so BASS_TRACE=1 yields exec_time_ns)."""
    import sys
    import types

    if "antenv.axon_hooks" in sys.modules:
        return
    holder = [None]
    mod = types.ModuleType("antenv.axon_hooks")
    mod.set_axon_ntff_profile_hook = lambda h: holder.__setitem__(0, h)
    mod.get_axon_ntff_profile_hook = lambda: holder[0]
    sys.modules["antenv.axon_hooks"] = mod
    try:
        import antenv

        antenv.axon_hooks = mod
        from trn_agent_boot.trn_boot import _ntff_profile_via_ctypes

        mod.set_axon_ntff_profile_hook(
            _ntff_profile_via_ctypes("/opt/axon/libaxon_pjrt.so")
        )
    except Exception:
        pass


def _n_tiles(C):
    """Split C into moving-operand tiles of width 256..512 (float32r needs
    >=256 columns per matmul for full PE rate; PSUM caps a tile at 512).
    A mid-width first tile gives the DMA ramp time to land the weight
    stream before tile 1's full-rate consumption starts; the smallest
    tile goes last to shorten the drain."""
    assert C >= 768
    sizes = [384]
    rem = C - 384
    while rem > 1024:
        sizes.append(512)
        rem -= 512
    if rem > 512:
        sizes.extend([512, rem - 512] if rem - 512 >= 256 else [rem - 256, 256])
    else:
        sizes.append(rem)
    assert sum(sizes) == C and all(256 <= s <= 512 for s in sizes[1:])
    return sizes


def _build(C, dt_name):
    import concourse.mybir as mybir
    from concourse import bacc
    from concourse.bass import ds
    from concourse.tile import TileContext

    dt_in = getattr(mybir.dt, dt_name)
    KT = D // P
    MT = D // P
    n_sizes = _n_tiles(C)
    NT = len(n_sizes)
    nc = bacc.Bacc(
        "TRN2", target_bir_lowering=False, debug=False, num_devices=N_CORES
    )
    # wp[j, ki, u, ko, o] = W_e[(2j+u)*128 + o, ko*128 + ki] — m-chunk PAIRS
    # are interleaved per-partition so each pair DMA moves 8KB contiguous
    # per-partition runs (bf16) instead of 4KB.
    wp = nc.dram_tensor("wp", [MT // 2, P, 2, KT, P], dt_in, kind="ExternalInput")
    # tokens pre-tiled on host: tile n is [P, KT, n_sz] with 2*KT*n_sz
    # contiguous bytes per partition (few, fat DMA descriptors).
    toks = [
        nc.dram_tensor(f"tok{n}", [P, KT, n_sizes[n]], dt_in, kind="ExternalInput")
        for n in range(NT)
    ]
    bias = nc.dram_tensor("bias", [D], mybir.dt.float32, kind="ExternalInput")
    # out2[m, o, t] = outT[m*128 + o, t]
    out2 = nc.dram_tensor("out2", [MT, P, C], dt_in, kind="ExternalOutput")

    b_r = bias.ap().rearrange("(mo mi) -> mi mo", mi=P)

    with TileContext(nc) as tc:
        with (
            tc.tile_pool(name="w", bufs=1) as w_pool,
            tc.tile_pool(name="tok", bufs=3) as tok_pool,
            tc.tile_pool(name="out", bufs=8) as out_pool,
            tc.tile_pool(name="bias", bufs=1) as b_pool,
            tc.tile_pool(name="ps", bufs=8, space="PSUM") as ps_pool,
        ):
            bias_tile = b_pool.tile([P, MT], mybir.dt.float32)
            nc.gpsimd.dma_start(bias_tile[:], b_r)

            tok_tiles = {}

            def load_toks(n, kchunks):
                n_sz = n_sizes[n]
                t_full = tok_pool.tile([P, KT, 512], dt_in, tag="tok")
                t_tile = t_full[:, :, :n_sz]
                # k-sliced chunks so the first matmuls of the tile only wait
                # for the slices they read, not the whole tile
                k = 0
                for kc in kchunks:
                    nc.sync.dma_start(
                        t_tile[:, k : k + kc, :],
                        toks[n].ap()[:, k : k + kc, :],
                    )
                    k += kc
                assert k == KT
                tok_tiles[n] = t_tile

            w_pairs = [None] * (MT // 2)

            def load_w(j, kchunks=(KT,), eng=None):
                w_t = w_pool.tile([P, 2, KT, P], dt_in, tag=f"w{j}")
                # most weights ride the Activation-HWDGE queue, but pairs
                # 1 and 3 ride SP-HWDGE (which has spare bandwidth early):
                # the DMA-ramp phase then feeds the PE from two hardware
                # queues in parallel. (The GpSimd queue is software-dynamic
                # and only boots ~15us in, too late to help.)
                eng = eng or nc.scalar
                k = 0
                for kc in kchunks:
                    eng.dma_start(
                        w_t[:, :, k : k + kc, :],
                        wp.ap()[j, :, :, k : k + kc, :],
                    )
                    k += kc
                assert k == KT
                w_pairs[j] = w_t

            def w_tile(m):
                return w_pairs[m // 2][:, m % 2]

            # issue order ~= consumption order: first k-chunks of the w0/w1
            # pair and tok0 land in ~2us so the PE starts immediately; the
            # rest of the weight stream follows in parallel with the token
            # stream.
            load_w(0, kchunks=(2, 2, 4, 8))
            load_toks(0, kchunks=(2, 2, 4, 8))
            load_w(1, kchunks=(4, 4, 8), eng=nc.sync)
            load_w(2, kchunks=(8, 8))
            load_w(3, eng=nc.sync)
            for j in range(4, MT // 2):
                load_w(j)

            for n in range(NT):
                n_sz = n_sizes[n]
                if n + 1 < NT:
                    load_toks(n + 1, kchunks=(8, 8))
                t_tile = tok_tiles.pop(n)
                for m in range(MT):
                    ps_full = ps_pool.tile([P, 512], mybir.dt.float32, tag="ps")
                    ps = ps_full[:, :n_sz]
                    wm = w_tile(m)
                    for k in range(KT):
                        nc.tensor.matmul(
                            ps,
                            wm[:, k, :],
                            t_tile[:, k, :],
                            start=(k == 0),
                            stop=(k == KT - 1),
                        )
                    o_full = out_pool.tile([P, 512], dt_in, tag="out")
                    o_tile = o_full[:, :n_sz]
                    nc.vector.tensor_scalar_add(
                        o_tile, ps, bias_tile[:, m : m + 1]
                    )
                    n_off = sum(n_sizes[:n])
                    nc.sync.dma_start(
                        out2.ap()[m, :, ds(n_off, n_sz)], o_tile
                    )
    nc.compile()
    return nc


def _get_program(C, dt_name):
    key = (C, dt_name)
    if key not in _cache:
        _cache[key] = _build(C, dt_name)
    return _cache[key]


# ------------------------------------------------------------------- host ---


def kernel(x, y, W_experts, b_experts, W_gate, b_gate):
    x = np.asarray(x, dtype=np.float32)
    y = np.asarray(y, dtype=np.float32)
    W_experts = np.asarray(W_experts, dtype=np.float32)
    b_experts = np.asarray(b_experts, dtype=np.float32)
    W_gate = np.asarray(W_gate, dtype=np.float32)
    b_gate = np.asarray(b_gate, dtype=np.float32)

    inp = np.concatenate([x, y], axis=1)  # [B, D]

    # ---- routing (host) ----
    logits = inp.astype(np.float64) @ W_gate.T.astype(np.float64) + b_gate
    order = np.argsort(-logits, axis=1, kind="stable")
    top2 = order[:, :TOPK]  # [B, 2]
    v = np.take_along_axis(logits, top2, axis=1)
    v = v - v.max(axis=1, keepdims=True)
    ev = np.exp(v)
    g = (ev / ev.sum(axis=1, keepdims=True)).astype(np.float32)  # [B, 2]

    # capacity factor 1.0: the device processes exactly B*K/E tokens per
    # expert; the few overflow tokens of hot experts (~2% here) are
    # handled on the host in fp32 while the device runs.
    C = B * TOPK // E  # 2048

    idx_list = []
    wgt_list = []
    ovf_idx = []
    ovf_wgt = []
    for e in range(E):
        m0 = top2[:, 0] == e
        m1 = top2[:, 1] == e
        idx_e = np.concatenate([np.nonzero(m0)[0], np.nonzero(m1)[0]])
        w_e = np.concatenate([g[m0, 0], g[m1, 1]])
        idx_list.append(idx_e[:C])
        wgt_list.append(w_e[:C])
        ovf_idx.append(idx_e[C:])
        ovf_wgt.append(w_e[C:])

    dt_name = os.environ.get("MOE_DT", "bfloat16")
    if dt_name == "bfloat16":
        import ml_dtypes

        np_in_dt = np.dtype(ml_dtypes.bfloat16)
    else:
        np_in_dt = np.dtype(np.float32)

    n_sizes = _n_tiles(C)
    inpT = np.ascontiguousarray(inp.T.astype(np_in_dt))  # [D, C] source
    MT = KT = D // P
    in_maps = []
    for e in range(E):
        toksT = np.zeros((D, C), dtype=np_in_dt)
        toksT[:, : len(idx_list[e])] = inpT[:, idx_list[e]]
        # tile n: [P, KT, n_sz] with tok_t[p, k, j] = toksT[k*128+p, off+j]
        t3 = toksT.reshape(KT, P, C).transpose(1, 0, 2)  # [P, KT, C]
        im = {}
        off = 0
        for n, n_sz in enumerate(n_sizes):
            im[f"tok{n}"] = np.ascontiguousarray(t3[:, :, off : off + n_sz])
            off += n_sz
        # wp[j, ki, u, ko, o] = W_e[(2j+u)*128 + o, ko*128 + ki]
        im["wp"] = np.ascontiguousarray(
            W_experts[e]
            .reshape(MT // 2, 2, P, KT, P)
            .transpose(0, 4, 1, 3, 2)
            .astype(np_in_dt)
        )
        im["bias"] = b_experts[e]
        in_maps.append(im)

    # ---- device ----
    if os.environ.get("BASS_TRACE"):
        _ntff_shim()
    from concourse.bass_utils import run_bass_kernel_spmd

    nc = _get_program(C, dt_name)
    res = None
    for attempt in range(3):
        try:
            res = run_bass_kernel_spmd(nc, in_maps, core_ids=list(range(N_CORES)))
            break
        except Exception:
            # the axon-tunneled device occasionally reports a transient
            # NRT_EXEC_UNIT_UNRECOVERABLE; it recovers after a short wait
            if attempt == 2:
                raise
            import time

            time.sleep(20 * (attempt + 1))
            try:
                import jax

                jax.clear_caches()
            except Exception:
                pass
    globals()["_last_res"] = res
    if res.exec_time_ns is not None:
        print(f"HW exec time: {res.exec_time_ns} ns")

    # ---- combine (host) ----
    fused = np.zeros((B, D), dtype=np.float32)
    for e in range(E):
        n_e = len(idx_list[e])
        if n_e == 0:
            continue
        outT = np.asarray(res.results[e]["out2"]).reshape(D, C).astype(np.float32)
        fused[idx_list[e]] += outT[:, :n_e].T * wgt_list[e][:, None]
        if len(ovf_idx[e]):
            o = inp[ovf_idx[e]] @ W_experts[e].T + b_experts[e]
            fused[ovf_idx[e]] += o * ovf_wgt[e][:, None]
    return fused


# revision 22
# speedup vs baseline: 1.0083x; 1.0083x over previous
"""BalancedMoE (B=8192, D=2048, E=8, top-2) on 8 Trainium2 NeuronCores.

Strategy: expert-parallel with host-side sparse dispatch.
  - Host computes gate logits / top-2 routing / softmax gates, gathers each
    expert's tokens into a k-tiled [128, KT, C] layout (contiguous 16KB
    per-partition DMA runs), in bf16.
  - Core e runs a dense [C, D] x [D, D] matmul for expert e only
    (top-2 of 8 experts => 4x less FLOPs than the dense reference),
    with the expert weight matrix stationary in SBUF.
  - Host scatters the per-expert outputs back and combines with the
    gate weights in fp32.

Per-core Bass kernel: outT[o, t] = sum_d W_e[o, d] * toks[t, d] + b_e[o]
  lhsT = W_e^T tiles (stationary), rhs = token tiles (moving).

bf16 inputs halve HBM traffic vs fp32 (PE rate is identical: 1 cycle/row
for both bf16 and float32r at >=256 columns); the DMA-bound startup and
weight-stream phases shrink accordingly. Accumulation stays fp32 in PSUM.
"""

import os

import numpy as np

P = 128
B = 8192
D_LAT = 1024
D_EMB = 1024
D = D_LAT + D_EMB  # 2048
E = 8
TOPK = 2
N_CORES = 8


# ----------------------------------------------------------------- device ---

_cache = {}


def _ntff_shim():
    """Register the axon NTFF profile hook that the boot skips when
    antenv.axon_hooks is missing (so BASS_TRACE=1 yields exec_time_ns)."""
    import sys
    import types

    if "antenv.axon_hooks" in sys.modules:
        return
    holder = [None]
    mod = types.ModuleType("antenv.axon_hooks")
    mod.set_axon_ntff_profile_hook = lambda h: holder.__setitem__(0, h)
    mod.get_axon_ntff_profile_hook = lambda: holder[0]
    sys.modules["antenv.axon_hooks"] = mod
    try:
        import antenv

        antenv.axon_hooks = mod
        from trn_agent_boot.trn_boot import _ntff_profile_via_ctypes

        mod.set_axon_ntff_profile_hook(
            _ntff_profile_via_ctypes("/opt/axon/libaxon_pjrt.so")
        )
    except Exception:
        pass


def _n_tiles(C):
    """Split C into moving-operand tiles of width 256..512 (float32r needs
    >=256 columns per matmul for full PE rate; PSUM caps a tile at 512).
    A mid-width first tile gives the DMA ramp time to land the weight
    stream before tile 1's full-rate consumption starts; the smallest
    tile goes last to shorten the drain."""
    assert C >= 768
    sizes = [384]
    rem = C - 384
    while rem > 1024:
        sizes.append(512)
        rem -= 512
    if rem > 512:
        sizes.extend([512, rem - 512] if rem - 512 >= 256 else [rem - 256, 256])
    else:
        sizes.append(rem)
    assert sum(sizes) == C and all(256 <= s <= 512 for s in sizes[1:])
    return sizes


def _build(C, dt_name):
    import concourse.mybir as mybir
    from concourse import bacc
    from concourse.bass import ds
    from concourse.tile import TileContext

    dt_in = getattr(mybir.dt, dt_name)
    KT = D // P
    MT = D // P
    n_sizes = _n_tiles(C)
    NT = len(n_sizes)
    nc = bacc.Bacc(
        "TRN2", target_bir_lowering=False, debug=False, num_devices=N_CORES
    )
    # wp[j, ki, u, ko, o] = W_e[(2j+u)*128 + o, ko*128 + ki] — m-chunk PAIRS
    # are interleaved per-partition so each pair DMA moves 8KB contiguous
    # per-partition runs (bf16) instead of 4KB.
    wp = nc.dram_tensor("wp", [MT // 2, P, 2, KT, P], dt_in, kind="ExternalInput")
    # tokens pre-tiled on host: tile n is [P, KT, n_sz] with 2*KT*n_sz
    # contiguous bytes per partition (few, fat DMA descriptors).
    toks = [
        nc.dram_tensor(f"tok{n}", [P, KT, n_sizes[n]], dt_in, kind="ExternalInput")
        for n in range(NT)
    ]
    bias = nc.dram_tensor("bias", [D], mybir.dt.float32, kind="ExternalInput")
    # out2[m, o, t] = outT[m*128 + o, t]
    out2 = nc.dram_tensor("out2", [MT, P, C], dt_in, kind="ExternalOutput")

    b_r = bias.ap().rearrange("(mo mi) -> mi mo", mi=P)

    with TileContext(nc) as tc:
        with (
            tc.tile_pool(name="w", bufs=1) as w_pool,
            tc.tile_pool(name="tok", bufs=2) as tok_pool,
            tc.tile_pool(name="out", bufs=6) as out_pool,
            tc.tile_pool(name="bias", bufs=1) as b_pool,
            tc.tile_pool(name="ps", bufs=8, space="PSUM") as ps_pool,
        ):
            bias_tile = b_pool.tile([P, MT], mybir.dt.float32)
            nc.gpsimd.dma_start(bias_tile[:], b_r)

            tok_tiles = {}

            def load_toks(n, kchunk):
                n_sz = n_sizes[n]
                t_full = tok_pool.tile([P, KT, 512], dt_in, tag="tok")
                t_tile = t_full[:, :, :n_sz]
                # k-sliced chunks so the first matmuls of the tile only wait
                # for the slices they read, not the whole tile
                for k in range(0, KT, kchunk):
                    nc.sync.dma_start(
                        t_tile[:, k : k + kchunk, :],
                        toks[n].ap()[:, k : k + kchunk, :],
                    )
                tok_tiles[n] = t_tile

            w_pairs = [None] * (MT // 2)

            def load_w(j, kchunk=KT):
                w_t = w_pool.tile([P, 2, KT, P], dt_in, tag=f"w{j}")
                # weights ride the Activation-HWDGE queue; tokens and
                # outputs ride SP-HWDGE, so the weight stream never
                # interleaves with the token/output stream. (The GpSimd
                # queue is software-dynamic and only boots ~15us in, too
                # late for the early weight pairs.)
                for k in range(0, KT, kchunk):
                    nc.scalar.dma_start(
                        w_t[:, :, k : k + kchunk, :],
                        wp.ap()[j, :, :, k : k + kchunk, :],
                    )
                w_pairs[j] = w_t

            def w_tile(m):
                return w_pairs[m // 2][:, m % 2]

            # issue order ~= consumption order: first k-chunks of the w0/w1
            # pair and tok0 land in ~2us so the PE starts immediately; the
            # rest of the weight stream follows in parallel with the token
            # stream.
            load_w(0, kchunk=4)
            load_toks(0, kchunk=4)
            load_w(1, kchunk=4)
            load_w(2, kchunk=8)
            for j in range(3, MT // 2):
                load_w(j)

            for n in range(NT):
                n_sz = n_sizes[n]
                if n + 1 < NT:
                    load_toks(n + 1, kchunk=8)
                t_tile = tok_tiles.pop(n)
                for m in range(MT):
                    ps_full = ps_pool.tile([P, 512], mybir.dt.float32, tag="ps")
                    ps = ps_full[:, :n_sz]
                    wm = w_tile(m)
                    for k in range(KT):
                        nc.tensor.matmul(
                            ps,
                            wm[:, k, :],
                            t_tile[:, k, :],
                            start=(k == 0),
                            stop=(k == KT - 1),
                        )
                    o_full = out_pool.tile([P, 512], dt_in, tag="out")
                    o_tile = o_full[:, :n_sz]
                    nc.vector.tensor_scalar_add(
                        o_tile, ps, bias_tile[:, m : m + 1]
                    )
                    n_off = sum(n_sizes[:n])
                    nc.sync.dma_start(
                        out2.ap()[m, :, ds(n_off, n_sz)], o_tile
                    )
    nc.compile()
    return nc


def _get_program(C, dt_name):
    key = (C, dt_name)
    if key not in _cache:
        _cache[key] = _build(C, dt_name)
    return _cache[key]


# ------------------------------------------------------------------- host ---


def kernel(x, y, W_experts, b_experts, W_gate, b_gate):
    x = np.asarray(x, dtype=np.float32)
    y = np.asarray(y, dtype=np.float32)
    W_experts = np.asarray(W_experts, dtype=np.float32)
    b_experts = np.asarray(b_experts, dtype=np.float32)
    W_gate = np.asarray(W_gate, dtype=np.float32)
    b_gate = np.asarray(b_gate, dtype=np.float32)

    inp = np.concatenate([x, y], axis=1)  # [B, D]

    # ---- routing (host) ----
    logits = inp.astype(np.float64) @ W_gate.T.astype(np.float64) + b_gate
    order = np.argsort(-logits, axis=1, kind="stable")
    top2 = order[:, :TOPK]  # [B, 2]
    v = np.take_along_axis(logits, top2, axis=1)
    v = v - v.max(axis=1, keepdims=True)
    ev = np.exp(v)
    g = (ev / ev.sum(axis=1, keepdims=True)).astype(np.float32)  # [B, 2]

    # capacity factor 1.0: the device processes exactly B*K/E tokens per
    # expert; the few overflow tokens of hot experts (~2% here) are
    # handled on the host in fp32 while the device runs.
    C = B * TOPK // E  # 2048

    idx_list = []
    wgt_list = []
    ovf_idx = []
    ovf_wgt = []
    for e in range(E):
        m0 = top2[:, 0] == e
        m1 = top2[:, 1] == e
        idx_e = np.concatenate([np.nonzero(m0)[0], np.nonzero(m1)[0]])
        w_e = np.concatenate([g[m0, 0], g[m1, 1]])
        idx_list.append(idx_e[:C])
        wgt_list.append(w_e[:C])
        ovf_idx.append(idx_e[C:])
        ovf_wgt.append(w_e[C:])

    dt_name = os.environ.get("MOE_DT", "bfloat16")
    if dt_name == "bfloat16":
        import ml_dtypes

        np_in_dt = np.dtype(ml_dtypes.bfloat16)
    else:
        np_in_dt = np.dtype(np.float32)

    n_sizes = _n_tiles(C)
    inpT = np.ascontiguousarray(inp.T.astype(np_in_dt))  # [D, C] source
    MT = KT = D // P
    in_maps = []
    for e in range(E):
        toksT = np.zeros((D, C), dtype=np_in_dt)
        toksT[:, : len(idx_list[e])] = inpT[:, idx_list[e]]
        # tile n: [P, KT, n_sz] with tok_t[p, k, j] = toksT[k*128+p, off+j]
        t3 = toksT.reshape(KT, P, C).transpose(1, 0, 2)  # [P, KT, C]
        im = {}
        off = 0
        for n, n_sz in enumerate(n_sizes):
            im[f"tok{n}"] = np.ascontiguousarray(t3[:, :, off : off + n_sz])
            off += n_sz
        # wp[j, ki, u, ko, o] = W_e[(2j+u)*128 + o, ko*128 + ki]
        im["wp"] = np.ascontiguousarray(
            W_experts[e]
            .reshape(MT // 2, 2, P, KT, P)
            .transpose(0, 4, 1, 3, 2)
            .astype(np_in_dt)
        )
        im["bias"] = b_experts[e]
        in_maps.append(im)

    # ---- device ----
    if os.environ.get("BASS_TRACE"):
        _ntff_shim()
    from concourse.bass_utils import run_bass_kernel_spmd

    nc = _get_program(C, dt_name)
    res = None
    for attempt in range(3):
        try:
            res = run_bass_kernel_spmd(nc, in_maps, core_ids=list(range(N_CORES)))
            break
        except Exception:
            # the axon-tunneled device occasionally reports a transient
            # NRT_EXEC_UNIT_UNRECOVERABLE; it recovers after a short wait
            if attempt == 2:
                raise
            import time

            time.sleep(20 * (attempt + 1))
            try:
                import jax

                jax.clear_caches()
            except Exception:
                pass
    globals()["_last_res"] = res
    if res.exec_time_ns is not None:
        print(f"HW exec time: {res.exec_time_ns} ns")

    # ---- combine (host) ----
    fused = np.zeros((B, D), dtype=np.float32)
    for e in range(E):
        n_e = len(idx_list[e])
        if n_e == 0:
            continue
        outT = np.asarray(res.results[e]["out2"]).reshape(D, C).astype(np.float32)
        fused[idx_list[e]] += outT[:, :n_e].T * wgt_list[e][:, None]
        if len(ovf_idx[e]):
            o = inp[ovf_idx[e]] @ W_experts[e].T + b_experts[e]
            fused[ovf_idx[e]] += o * ovf_wgt[e][:, None]
    return fused


# revision 23
# speedup vs baseline: 1.0219x; 1.0135x over previous
"""BalancedMoE (B=8192, D=2048, E=8, top-2) on 8 Trainium2 NeuronCores.

Strategy: expert-parallel with host-side sparse dispatch.
  - Host computes gate logits / top-2 routing / softmax gates, gathers each
    expert's tokens into a k-tiled [128, KT, C] layout (contiguous 16KB
    per-partition DMA runs), in bf16.
  - Core e runs a dense [C, D] x [D, D] matmul for expert e only
    (top-2 of 8 experts => 4x less FLOPs than the dense reference),
    with the expert weight matrix stationary in SBUF.
  - Host scatters the per-expert outputs back and combines with the
    gate weights in fp32.

Per-core Bass kernel: outT[o, t] = sum_d W_e[o, d] * toks[t, d] + b_e[o]
  lhsT = W_e^T tiles (stationary), rhs = token tiles (moving).

bf16 inputs halve HBM traffic vs fp32 (PE rate is identical: 1 cycle/row
for both bf16 and float32r at >=256 columns); the DMA-bound startup and
weight-stream phases shrink accordingly. Accumulation stays fp32 in PSUM.
"""

import os

import numpy as np

P = 128
B = 8192
D_LAT = 1024
D_EMB = 1024
D = D_LAT + D_EMB  # 2048
E = 8
TOPK = 2
N_CORES = 8


# ----------------------------------------------------------------- device ---

_cache = {}


def _ntff_shim():
    """Register the axon NTFF profile hook that the boot skips when
    antenv.axon_hooks is missing (so BASS_TRACE=1 yields exec_time_ns)."""
    import sys
    import types

    if "antenv.axon_hooks" in sys.modules:
        return
    holder = [None]
    mod = types.ModuleType("antenv.axon_hooks")
    mod.set_axon_ntff_profile_hook = lambda h: holder.__setitem__(0, h)
    mod.get_axon_ntff_profile_hook = lambda: holder[0]
    sys.modules["antenv.axon_hooks"] = mod
    try:
        import antenv

        antenv.axon_hooks = mod
        from trn_agent_boot.trn_boot import _ntff_profile_via_ctypes

        mod.set_axon_ntff_profile_hook(
            _ntff_profile_via_ctypes("/opt/axon/libaxon_pjrt.so")
        )
    except Exception:
        pass


def _n_tiles(C):
    """Split C into moving-operand tiles of width 256..512 (float32r needs
    >=256 columns per matmul for full PE rate; PSUM caps a tile at 512).
    Full 512-wide tiles minimize the matmul count and keep the first
    tile's weight-demand rate below what the ramping DMA can supply."""
    assert C >= 768
    sizes = [512] * (C // 512)
    rem = C % 512
    if rem >= 256:
        sizes.append(rem)
    elif rem:
        sizes[-1] -= 256 - rem
        sizes.append(256)
    assert sum(sizes) == C and all(256 <= s <= 512 for s in sizes)
    return sizes


def _build(C, dt_name):
    import concourse.mybir as mybir
    from concourse import bacc
    from concourse.bass import ds
    from concourse.tile import TileContext

    dt_in = getattr(mybir.dt, dt_name)
    KT = D // P
    MT = D // P
    n_sizes = _n_tiles(C)
    NT = len(n_sizes)
    nc = bacc.Bacc(
        "TRN2", target_bir_lowering=False, debug=False, num_devices=N_CORES
    )
    # wp[j, ki, u, ko, o] = W_e[(2j+u)*128 + o, ko*128 + ki] — m-chunk PAIRS
    # are interleaved per-partition so each pair DMA moves 8KB contiguous
    # per-partition runs (bf16) instead of 4KB.
    wp = nc.dram_tensor("wp", [MT // 2, P, 2, KT, P], dt_in, kind="ExternalInput")
    # tokens pre-tiled on host: tile n is [P, KT, n_sz] with 2*KT*n_sz
    # contiguous bytes per partition (few, fat DMA descriptors).
    toks = [
        nc.dram_tensor(f"tok{n}", [P, KT, n_sizes[n]], dt_in, kind="ExternalInput")
        for n in range(NT)
    ]
    bias = nc.dram_tensor("bias", [D], mybir.dt.float32, kind="ExternalInput")
    # out2[m, o, t] = outT[m*128 + o, t]
    out2 = nc.dram_tensor("out2", [MT, P, C], dt_in, kind="ExternalOutput")

    b_r = bias.ap().rearrange("(mo mi) -> mi mo", mi=P)

    with TileContext(nc) as tc:
        with (
            tc.tile_pool(name="w", bufs=1) as w_pool,
            tc.tile_pool(name="tok", bufs=2) as tok_pool,
            tc.tile_pool(name="out", bufs=6) as out_pool,
            tc.tile_pool(name="bias", bufs=1) as b_pool,
            tc.tile_pool(name="ps", bufs=8, space="PSUM") as ps_pool,
        ):
            bias_tile = b_pool.tile([P, MT], mybir.dt.float32)
            nc.gpsimd.dma_start(bias_tile[:], b_r)

            tok_tiles = {}

            def load_toks(n, kchunk):
                n_sz = n_sizes[n]
                t_full = tok_pool.tile([P, KT, 512], dt_in, tag="tok")
                t_tile = t_full[:, :, :n_sz]
                # k-sliced chunks so the first matmuls of the tile only wait
                # for the slices they read, not the whole tile
                for k in range(0, KT, kchunk):
                    nc.sync.dma_start(
                        t_tile[:, k : k + kchunk, :],
                        toks[n].ap()[:, k : k + kchunk, :],
                    )
                tok_tiles[n] = t_tile

            w_pairs = [None] * (MT // 2)

            def load_w(j, kchunk=KT):
                w_t = w_pool.tile([P, 2, KT, P], dt_in, tag=f"w{j}")
                # weights ride the Activation-HWDGE queue; tokens and
                # outputs ride SP-HWDGE, so the weight stream never
                # interleaves with the token/output stream. (The GpSimd
                # queue is software-dynamic and only boots ~15us in, too
                # late for the early weight pairs.)
                for k in range(0, KT, kchunk):
                    nc.scalar.dma_start(
                        w_t[:, :, k : k + kchunk, :],
                        wp.ap()[j, :, :, k : k + kchunk, :],
                    )
                w_pairs[j] = w_t

            def w_tile(m):
                return w_pairs[m // 2][:, m % 2]

            # issue order ~= consumption order: first k-chunks of the w0/w1
            # pair and tok0 land in ~2us so the PE starts immediately; the
            # rest of the weight stream follows in parallel with the token
            # stream.
            load_w(0, kchunk=4)
            load_toks(0, kchunk=4)
            load_w(1, kchunk=4)
            load_w(2, kchunk=8)
            for j in range(3, MT // 2):
                load_w(j)

            for n in range(NT):
                n_sz = n_sizes[n]
                if n + 1 < NT:
                    load_toks(n + 1, kchunk=8)
                t_tile = tok_tiles.pop(n)
                for m in range(MT):
                    ps_full = ps_pool.tile([P, 512], mybir.dt.float32, tag="ps")
                    ps = ps_full[:, :n_sz]
                    wm = w_tile(m)
                    for k in range(KT):
                        nc.tensor.matmul(
                            ps,
                            wm[:, k, :],
                            t_tile[:, k, :],
                            start=(k == 0),
                            stop=(k == KT - 1),
                        )
                    o_full = out_pool.tile([P, 512], dt_in, tag="out")
                    o_tile = o_full[:, :n_sz]
                    nc.vector.tensor_scalar_add(
                        o_tile, ps, bias_tile[:, m : m + 1]
                    )
                    n_off = sum(n_sizes[:n])
                    nc.sync.dma_start(
                        out2.ap()[m, :, ds(n_off, n_sz)], o_tile
                    )
    nc.compile()
    return nc


def _get_program(C, dt_name):
    key = (C, dt_name)
    if key not in _cache:
        _cache[key] = _build(C, dt_name)
    return _cache[key]


# ------------------------------------------------------------------- host ---


def kernel(x, y, W_experts, b_experts, W_gate, b_gate):
    x = np.asarray(x, dtype=np.float32)
    y = np.asarray(y, dtype=np.float32)
    W_experts = np.asarray(W_experts, dtype=np.float32)
    b_experts = np.asarray(b_experts, dtype=np.float32)
    W_gate = np.asarray(W_gate, dtype=np.float32)
    b_gate = np.asarray(b_gate, dtype=np.float32)

    inp = np.concatenate([x, y], axis=1)  # [B, D]

    # ---- routing (host) ----
    logits = inp.astype(np.float64) @ W_gate.T.astype(np.float64) + b_gate
    order = np.argsort(-logits, axis=1, kind="stable")
    top2 = order[:, :TOPK]  # [B, 2]
    v = np.take_along_axis(logits, top2, axis=1)
    v = v - v.max(axis=1, keepdims=True)
    ev = np.exp(v)
    g = (ev / ev.sum(axis=1, keepdims=True)).astype(np.float32)  # [B, 2]

    # capacity factor 1.0: the device processes exactly B*K/E tokens per
    # expert; the few overflow tokens of hot experts (~2% here) are
    # handled on the host in fp32 while the device runs.
    C = B * TOPK // E  # 2048

    idx_list = []
    wgt_list = []
    ovf_idx = []
    ovf_wgt = []
    for e in range(E):
        m0 = top2[:, 0] == e
        m1 = top2[:, 1] == e
        idx_e = np.concatenate([np.nonzero(m0)[0], np.nonzero(m1)[0]])
        w_e = np.concatenate([g[m0, 0], g[m1, 1]])
        idx_list.append(idx_e[:C])
        wgt_list.append(w_e[:C])
        ovf_idx.append(idx_e[C:])
        ovf_wgt.append(w_e[C:])

    dt_name = os.environ.get("MOE_DT", "bfloat16")
    if dt_name == "bfloat16":
        import ml_dtypes

        np_in_dt = np.dtype(ml_dtypes.bfloat16)
    else:
        np_in_dt = np.dtype(np.float32)

    n_sizes = _n_tiles(C)
    inpT = np.ascontiguousarray(inp.T.astype(np_in_dt))  # [D, C] source
    MT = KT = D // P
    in_maps = []
    for e in range(E):
        toksT = np.zeros((D, C), dtype=np_in_dt)
        toksT[:, : len(idx_list[e])] = inpT[:, idx_list[e]]
        # tile n: [P, KT, n_sz] with tok_t[p, k, j] = toksT[k*128+p, off+j]
        t3 = toksT.reshape(KT, P, C).transpose(1, 0, 2)  # [P, KT, C]
        im = {}
        off = 0
        for n, n_sz in enumerate(n_sizes):
            im[f"tok{n}"] = np.ascontiguousarray(t3[:, :, off : off + n_sz])
            off += n_sz
        # wp[j, ki, u, ko, o] = W_e[(2j+u)*128 + o, ko*128 + ki]
        im["wp"] = np.ascontiguousarray(
            W_experts[e]
            .reshape(MT // 2, 2, P, KT, P)
            .transpose(0, 4, 1, 3, 2)
            .astype(np_in_dt)
        )
        im["bias"] = b_experts[e]
        in_maps.append(im)

    # ---- device ----
    if os.environ.get("BASS_TRACE"):
        _ntff_shim()
    from concourse.bass_utils import run_bass_kernel_spmd

    nc = _get_program(C, dt_name)
    res = None
    for attempt in range(3):
        try:
            res = run_bass_kernel_spmd(nc, in_maps, core_ids=list(range(N_CORES)))
            break
        except Exception:
            # the axon-tunneled device occasionally reports a transient
            # NRT_EXEC_UNIT_UNRECOVERABLE; it recovers after a short wait
            if attempt == 2:
                raise
            import time

            time.sleep(20 * (attempt + 1))
            try:
                import jax

                jax.clear_caches()
            except Exception:
                pass
    globals()["_last_res"] = res
    if res.exec_time_ns is not None:
        print(f"HW exec time: {res.exec_time_ns} ns")

    # ---- combine (host) ----
    fused = np.zeros((B, D), dtype=np.float32)
    for e in range(E):
        n_e = len(idx_list[e])
        if n_e == 0:
            continue
        outT = np.asarray(res.results[e]["out2"]).reshape(D, C).astype(np.float32)
        fused[idx_list[e]] += outT[:, :n_e].T * wgt_list[e][:, None]
        if len(ovf_idx[e]):
            o = inp[ovf_idx[e]] @ W_experts[e].T + b_experts[e]
            fused[ovf_idx[e]] += o * ovf_wgt[e][:, None]
    return fused
